# revision 1
# baseline (speedup 1.0000x reference)
"""CrossAttention Trainium2 kernel.

Full inputs -> shard over 8 cores (batch x head-group) -> Bass kernel ->
host gather (sum head-group partials per batch + bias).

Per-core layout (B=2 batches x 4 head-groups of 4 heads):
  xT    [1024, 2048]  x[b].T
  ctxT  [1024, 2048]  context[b].T
  wqT   [1024, 256]   Wq[rows(g)].T      (rows(g) = g*256 : (g+1)*256)
  wkT   [1024, 256]
  wvT   [1024, 256]
  woT   [256, 1024]   Wo[:, rows(g)].T
  out y [2048, 1024]  partial (sum over g gives batch output; bias on host)

All matmuls run fp32r (fp32 data, reduced-precision PE multiply, full rate
for N>=256). Scores are computed transposed (keys on partitions) so the
attention@V matmul needs no transposes; softmax denominators come from a
ones-column in the AV lhsT; normalization is broadcast along the free axis
via a K=128 matmul with a constant E matrix.
"""
import numpy as np
import ml_dtypes

HEADS = 16
DIM_HEAD = 64
D_MODEL = 1024
N_CORES = 8


def build_nc(n_q=2048, n_kv=2048, d_model=1024, n_heads=4, d_head=64, nt=512):
    """Build the per-core Bass module. Returns (nc, io_names)."""
    import concourse.bass as bass
    import concourse.mybir as mybir
    import concourse.tile as tile
    from concourse import bacc

    FP32 = mybir.dt.float32
    FP32R = mybir.dt.float32r
    BF16 = mybir.dt.bfloat16
    EXP = mybir.ActivationFunctionType.Exp
    P = 128

    inner = n_heads * d_head          # 256
    ND = d_model // P                 # d-chunks (8)
    NI = inner // P                   # i-chunks (2)
    NNT = n_q // nt                   # query tiles (4)
    NMT = n_kv // nt                  # key tiles for k-proj (4)
    NMC = n_kv // P                   # key chunks (16)
    NJ = d_model // nt                # output col tiles (2)
    NHP = n_heads // 2                # head pairs (2)
    assert NMC % 2 == 0
    MPAIRS = NMC // 2                 # m-chunk pairs (8)

    nc = bacc.Bacc(None, target_bir_lowering=False, debug=False)

    xT = nc.dram_tensor("xT", [d_model, n_q], BF16, kind="ExternalInput")
    ctxT = nc.dram_tensor("ctxT", [d_model, n_kv], BF16, kind="ExternalInput")
    wqT = nc.dram_tensor("wqT", [d_model, inner], BF16, kind="ExternalInput")
    wkT = nc.dram_tensor("wkT", [d_model, inner], BF16, kind="ExternalInput")
    wvT = nc.dram_tensor("wvT", [d_model, inner], BF16, kind="ExternalInput")
    woT = nc.dram_tensor("woT", [inner, d_model], FP32R, kind="ExternalInput")
    y = nc.dram_tensor("y", [n_q, d_model], FP32, kind="ExternalOutput")

    xT_r = xT.ap().rearrange("(c p) n -> p c n", p=P)      # [128, ND, n_q]
    ctxT_r = ctxT.ap().rearrange("(c p) m -> p c m", p=P)  # [128, ND, n_kv]
    wqT_r = wqT.ap().rearrange("(c p) i -> p c i", p=P)    # [128, ND, inner]
    wkT_r = wkT.ap().rearrange("(c p) i -> p c i", p=P)
    wvT_r = wvT.ap().rearrange("(c p) i -> p c i", p=P)
    woT_r = woT.ap().rearrange("(c p) j -> p c j", p=P)    # [128, NI, d_model]

    scale = float(d_head) ** -0.5

    with tile.TileContext(nc) as tc:
        with (
            tc.tile_pool(name="persist", bufs=1) as persist,
            tc.tile_pool(name="vpool", bufs=NMC) as vpool,
        ):
            # ---------------- persistent tiles ----------------
            qT_sb = [persist.tile([P, n_q], BF16, tag=f"qT{i}", name=f"qT{i}") for i in range(NI)]
            kT_sb = [persist.tile([P, n_kv], BF16, tag=f"kT{i}", name=f"kT{i}") for i in range(NI)]
            woT_sb = persist.tile([P, NI, d_model], FP32R, tag="woT")
            nc.sync.dma_start(woT_sb[:], woT_r[:, :, :])
            # E: broadcast matrix (row0 -> out rows 0..63, row64 -> 64..127)
            scratch = persist.tile([P, nt], FP32, tag="scratch")
            nc.vector.memset(scratch[:], 0.0)
            E_sb = persist.tile([P, P], FP32R, tag="E")
            nc.vector.tensor_copy(E_sb[:], scratch[:, 0:P])
            # rs: reciprocal softmax sums at rows 0/64; other rows stay 0
            # (the E-matmul reads all 128 partitions - keep them finite)
            rs_p = persist.tile([P, nt], FP32R, tag="rs")
            nc.vector.tensor_copy(rs_p[:], scratch[:])
            ones_sc = persist.tile([P, 65 * n_heads], FP32, tag="ones_sc")
            nc.vector.memset(ones_sc[:], 1.0)
            rs32 = persist.tile([P, nt], FP32, tag="rs32")
            nc.vector.memset(rs32[:], 0.0)
            # ssb rows 1..63 stay 1.0 forever: reciprocal of the whole
            # [0:65] block is then finite everywhere; E zeros mask them.
            ssb_p = persist.tile([P, nt], FP32, tag="ssb_p")
            nc.vector.memset(ssb_p[:], 1.0)
            nc.vector.tensor_copy(E_sb[0:1, 0:64], ones_sc[0:1, 0:64])
            nc.vector.tensor_copy(E_sb[64:65, 64:128], ones_sc[64:65, 0:64])
            # v tiles: per m-chunk [128, n_heads*65]; col h*65+64 is the
            # ones column (softmax denominator trick)
            v_sb = [vpool.tile([P, n_heads * 65], BF16, tag="vsb", name=f"vsb{m}")
                    for m in range(NMC)]

            # ---------------- stage 1: projections ----------------
            with (
                tc.tile_pool(name="s1w", bufs=1) as s1w,
                tc.tile_pool(name="s1x", bufs=2) as s1x,
                tc.tile_pool(name="s1ps", bufs=2, space="PSUM") as s1ps,
            ):
                wq_sb = s1w.tile([P, ND, inner], BF16, tag="wq")
                wk_sb = s1w.tile([P, ND, inner], BF16, tag="wk")
                wv_sb = s1w.tile([P, ND, inner], BF16, tag="wv")
                nc.sync.dma_start(wq_sb[:], wqT_r[:, :, :])
                nc.sync.dma_start(wk_sb[:], wkT_r[:, :, :])
                nc.sync.dma_start(wv_sb[:], wvT_r[:, :, :])

                # qT[i, n] accumulation over d
                for n in range(NNT):
                    xq = s1x.tile([P, ND, nt], BF16, tag="xq")
                    nc.sync.dma_start(xq[:], xT_r[:, :, n * nt:(n + 1) * nt])
                    for i in range(NI):
                        ps = s1ps.tile([P, nt], FP32, tag="qk")
                        for d in range(ND):
                            nc.tensor.matmul(
                                ps[:],
                                wq_sb[:, d, i * P:(i + 1) * P],
                                xq[:, d, :],
                                start=(d == 0), stop=(d == ND - 1))
                        nc.vector.tensor_copy(
                            qT_sb[i][:, n * nt:(n + 1) * nt], ps[:])
                # kT[i, m]
                for m in range(NMT):
                    ck = s1x.tile([P, ND, nt], BF16, tag="ck")
                    nc.sync.dma_start(ck[:], ctxT_r[:, :, m * nt:(m + 1) * nt])
                    for i in range(NI):
                        ps = s1ps.tile([P, nt], FP32, tag="qk")
                        for d in range(ND):
                            nc.tensor.matmul(
                                ps[:],
                                wk_sb[:, d, i * P:(i + 1) * P],
                                ck[:, d, :],
                                start=(d == 0), stop=(d == ND - 1))
                        nc.vector.tensor_copy(
                            kT_sb[i][:, m * nt:(m + 1) * nt], ps[:])
                # v[m, i]: lhsT = ctxT chunk cols, rhs = wvT
                for m in range(NMC):
                    cv = s1x.tile([P, ND, P], BF16, tag="cv")
                    nc.sync.dma_start(cv[:], ctxT_r[:, :, m * P:(m + 1) * P])
                    ps = s1ps.tile([P, inner], FP32, tag="vps")
                    for d in range(ND):
                        nc.tensor.matmul(
                            ps[:], cv[:, d, :],
                            wv_sb[:, d, :],
                            start=(d == 0), stop=(d == ND - 1))
                    # fill ones cols, then strided per-head evac
                    nc.vector.tensor_copy(v_sb[m][:], ones_sc[:])
                    for h in range(n_heads):
                        nc.vector.tensor_copy(
                            v_sb[m][:, h * 65:h * 65 + 64],
                            ps[:, h * d_head:(h + 1) * d_head])

            # ---------------- stages 2-4 ----------------
            with (
                tc.tile_pool(name="psq", bufs=2, space="PSUM") as psqp,
                tc.tile_pool(name="upool", bufs=2, space="PSUM") as upool,
                tc.tile_pool(name="mps", bufs=2, space="PSUM") as mps,
                tc.tile_pool(name="s2sb", bufs=2) as s2sb,
                tc.tile_pool(name="expp", bufs=4) as expp,
                tc.tile_pool(name="apool", bufs=2 * NI) as apool,
            ):
                for n in range(NNT):
                    nsl = slice(n * nt, (n + 1) * nt)
                    A_sb = [apool.tile([P, nt], FP32R, tag="A", name=f"A{n}_{c}") for c in range(NI)]
                    for hp in range(NHP):
                        h0, h1 = 2 * hp, 2 * hp + 1
                        U = [upool.tile([65, nt], FP32, tag="U", name=f"U{n}_{hp}_{u}") for u in range(2)]
                        for mp in range(MPAIRS):
                            pq0 = psqp.tile([P, 2 * nt], FP32, tag="psq")
                            pq1 = psqp.tile([P, 2 * nt], FP32, tag="psq")
                            for s in range(2):
                                m = 2 * mp + s
                                msl = slice(m * P, (m + 1) * P)
                                ssl = slice(s * nt, (s + 1) * nt)
                                # scoresT for h0 (rows 0:64) / h1 (rows 64:128),
                                # row-packed to run concurrently on the PE
                                nc.tensor.matmul(
                                    pq0[:, ssl],
                                    kT_sb[hp][0:64, msl],
                                    qT_sb[hp][0:64, nsl],
                                    start=True, stop=True, tile_position=(0, 0))
                                nc.tensor.matmul(
                                    pq1[:, ssl],
                                    kT_sb[hp][64:128, msl],
                                    qT_sb[hp][64:128, nsl],
                                    start=True, stop=True, tile_position=(64, 0))
                            ex0 = expp.tile([P, 2 * nt], BF16, tag="ex")
                            ex1 = expp.tile([P, 2 * nt], BF16, tag="ex")
                            nc.scalar.activation(ex0[:], pq0[:, :], EXP, scale=scale)
                            nc.scalar.activation(ex1[:], pq1[:, :], EXP, scale=scale)
                            for s in range(2):
                                m = 2 * mp + s
                                ssl = slice(s * nt, (s + 1) * nt)
                                first = (mp == 0 and s == 0)
                                last = (mp == MPAIRS - 1 and s == 1)
                                nc.tensor.matmul(
                                    U[0][:, :],
                                    v_sb[m][:, h0 * 65:h0 * 65 + 65],
                                    ex0[:, ssl],
                                    start=first, stop=last)
                                nc.tensor.matmul(
                                    U[1][:, :],
                                    v_sb[m][:, h1 * 65:h1 * 65 + 65],
                                    ex1[:, ssl],
                                    start=first, stop=last)
                        # normalization: A[c] rows 0:64 = U0[0:64]/S0, 64:128 = U1/S1
                        usb = s2sb.tile([P, nt], FP32, tag="usb")
                        nc.vector.tensor_copy(usb[0:64, :], U[0][0:64, :])
                        nc.vector.tensor_copy(usb[64:128, :], U[1][0:64, :])
                        nc.vector.tensor_copy(ssb_p[0:1, :], U[0][64:65, :])
                        nc.vector.tensor_copy(ssb_p[64:65, :], U[1][64:65, :])
                        with nc.allow_low_precision(reason="recip rows to f32"):
                            nc.vector.reciprocal(rs32[0:65, :], ssb_p[0:65, :])
                        nc.vector.tensor_copy(rs_p[0:65, :], rs32[0:65, :])
                        bps = mps.tile([P, nt], FP32, tag="m")
                        nc.tensor.matmul(bps[:], E_sb[:],
                                         rs_p[:],
                                         start=True, stop=True)
                        bsb = s2sb.tile([P, nt], FP32, tag="bsb")
                        nc.vector.tensor_copy(bsb[:], bps[:])
                        nc.vector.tensor_mul(A_sb[hp][0:64, :], usb[0:64, :],
                                             bsb[0:64, :])
                        nc.vector.tensor_mul(A_sb[hp][64:128, :], usb[64:128, :],
                                             bsb[64:128, :])
                    # stage 4: y[n-rows, :] = A.T @ woT
                    for q in range(nt // P):
                        qsl = slice(q * P, (q + 1) * P)
                        for j in range(NJ):
                            jsl = slice(j * nt, (j + 1) * nt)
                            yps = mps.tile([P, nt], FP32, tag="m")
                            for c in range(NI):
                                nc.tensor.matmul(
                                    yps[:], A_sb[c][:, qsl],
                                    woT_sb[:, c, jsl],
                                    start=(c == 0), stop=(c == NI - 1))
                            ysb = s2sb.tile([P, nt], FP32, tag="ysb")
                            nc.vector.tensor_copy(ysb[:], yps[:])
                            nc.sync.dma_start(
                                y.ap()[n * nt + q * P:n * nt + (q + 1) * P, jsl],
                                ysb[:])

    nc.compile()
    return nc


def shard_inputs(x, context, Wq, Wk, Wv, Wo):
    """Per-core input dicts: core c -> (batch c//4, head-group c%4)."""
    in_maps = []
    for c in range(N_CORES):
        b, g = c // 4, c % 4
        rows = slice(g * 256, (g + 1) * 256)
        bf = ml_dtypes.bfloat16
        in_maps.append({
            "xT": np.ascontiguousarray(x[b].T).astype(bf),
            "ctxT": np.ascontiguousarray(context[b].T).astype(bf),
            "wqT": np.ascontiguousarray(Wq[rows].T).astype(bf),
            "wkT": np.ascontiguousarray(Wk[rows].T).astype(bf),
            "wvT": np.ascontiguousarray(Wv[rows].T).astype(bf),
            "woT": np.ascontiguousarray(Wo[:, rows].T),
        })
    return in_maps


_CACHE = {}


def _get_nc():
    if "nc" not in _CACHE:
        _CACHE["nc"] = build_nc()
    return _CACHE["nc"]


def kernel(x, context, Wq, Wk, Wv, Wo, bo, _trace=False):
    from concourse.bass_utils import run_bass_kernel_spmd

    x = np.asarray(x, dtype=np.float32)
    context = np.asarray(context, dtype=np.float32)
    in_maps = shard_inputs(x, context,
                           np.asarray(Wq, np.float32), np.asarray(Wk, np.float32),
                           np.asarray(Wv, np.float32), np.asarray(Wo, np.float32))
    nc = _get_nc()
    res = run_bass_kernel_spmd(nc, in_maps, core_ids=list(range(N_CORES)),
                               trace=_trace)
    B, N, _ = x.shape
    out = np.zeros((B, N, D_MODEL), dtype=np.float32)
    for c in range(N_CORES):
        out[c // 4] += res.results[c]["y"]
    out += np.asarray(bo, np.float32)[None, None, :]
    if _trace:
        _CACHE["last_results"] = res
    return out



# revision 3
# speedup vs baseline: 1.4443x; 1.4443x over previous
"""CrossAttention Trainium2 kernel (v2 — pipelined).

Full inputs -> shard over 8 cores (batch x head-group) -> Bass kernel ->
host gather (sum head-group partials per batch + bias).

Per-core layout (B=2 batches x 4 head-groups of 4 heads):
  xT    [1024, 2048]  x[b].T
  ctxT  [1024, 2048]  context[b].T
  wqT   [1024, 256]   Wq[rows(g)].T      (rows(g) = g*256 : (g+1)*256)
  wkT   [1024, 256]
  wvT   [1024, 256]
  woT   [256, 1024]   Wo[:, rows(g)].T
  out y [2048, 1024]  partial (sum over g gives batch output; bias on host)

v2 structure: the scalar engine's exp stream (128 ACTIVATE of 128x1024,
~147us) is the roofline; everything else is scheduled to hide under it.
- k/v projections share one ctx-tile load (ctx read once, not twice) and
  are interleaved with n-tile 0's score/exp/AV chunks so the exp stream
  starts ~15us in instead of after a 100us serial projection phase.
- exp table preloaded at t=0 via a dummy activation (overlaps input DMA).
- qT stored as per-(i,n) tiles (no WAR aliasing with next-tile scores);
  q-proj for n+1 and stage-4 for n-1 fill PE slack inside n's exp window.
- PSUM budget exactly 8 banks: scores 2x[128,1024](4) + U 2x[65,512](2)
  + one shared 2-slot transient pool (proj/E/stage4 outputs).
- softmax denominators via reciprocal_approx_fast (18-bit, ~5x faster).
"""
import numpy as np
import ml_dtypes

HEADS = 16
DIM_HEAD = 64
D_MODEL = 1024
N_CORES = 8


def build_nc(n_q=2048, n_kv=2048, d_model=1024, n_heads=4, d_head=64, nt=512):
    """Build the per-core Bass module."""
    import concourse.bass as bass
    import concourse.mybir as mybir
    import concourse.tile as tile
    from concourse import bacc

    FP32 = mybir.dt.float32
    FP32R = mybir.dt.float32r
    BF16 = mybir.dt.bfloat16
    EXP = mybir.ActivationFunctionType.Exp
    P = 128

    inner = n_heads * d_head          # 256
    ND = d_model // P                 # d-chunks (8)
    NI = inner // P                   # i-chunks (2)
    NNT = n_q // nt                   # query tiles (4)
    NMT = n_kv // nt                  # key tiles (4)
    NMC = n_kv // P                   # key chunks (16)
    NJ = d_model // nt                # output col tiles (2)
    NHP = n_heads // 2                # head pairs (2)
    MPAIRS = NMC // 2                 # m-chunk pairs (8)

    nc = bacc.Bacc(None, target_bir_lowering=False, debug=False)

    xT = nc.dram_tensor("xT", [d_model, n_q], BF16, kind="ExternalInput")
    ctxT = nc.dram_tensor("ctxT", [d_model, n_kv], BF16, kind="ExternalInput")
    wqT = nc.dram_tensor("wqT", [d_model, inner], BF16, kind="ExternalInput")
    wkT = nc.dram_tensor("wkT", [d_model, inner], BF16, kind="ExternalInput")
    wvT = nc.dram_tensor("wvT", [d_model, inner], BF16, kind="ExternalInput")
    woT = nc.dram_tensor("woT", [inner, d_model], FP32R, kind="ExternalInput")
    y = nc.dram_tensor("y", [n_q, d_model], FP32, kind="ExternalOutput")

    xT_r = xT.ap().rearrange("(c p) n -> p c n", p=P)      # [128, ND, n_q]
    ctxT_r = ctxT.ap().rearrange("(c p) m -> p c m", p=P)  # [128, ND, n_kv]
    wqT_r = wqT.ap().rearrange("(c p) i -> p c i", p=P)    # [128, ND, inner]
    wkT_r = wkT.ap().rearrange("(c p) i -> p c i", p=P)
    wvT_r = wvT.ap().rearrange("(c p) i -> p c i", p=P)
    woT_r = woT.ap().rearrange("(c p) j -> p c j", p=P)    # [128, NI, d_model]

    scale = float(d_head) ** -0.5

    with tile.TileContext(nc) as tc:
        with (
            tc.tile_pool(name="persist", bufs=1) as persist,
            tc.tile_pool(name="ctxp", bufs=3) as ctxp,
            tc.tile_pool(name="xqp", bufs=2) as xqp,
            tc.tile_pool(name="expp", bufs=6) as expp,
            tc.tile_pool(name="apool", bufs=2 * NI) as apool,
            tc.tile_pool(name="ysbp", bufs=3) as ysbp,
            tc.tile_pool(name="normp", bufs=2) as normp,
            tc.tile_pool(name="psq", bufs=2, space="PSUM") as psqp,
            tc.tile_pool(name="upool", bufs=2, space="PSUM") as upool,
            tc.tile_pool(name="trans", bufs=2, space="PSUM") as trans,
        ):
            # ---------------- persistent tiles ----------------
            qT = [[persist.tile([P, nt], BF16, tag=f"qT{i}_{n}", name=f"qT{i}_{n}")
                   for n in range(NNT)] for i in range(NI)]
            kT = [persist.tile([P, n_kv], BF16, tag=f"kT{i}", name=f"kT{i}") for i in range(NI)]
            v_sb = [persist.tile([P, n_heads * 65], BF16, tag=f"v{m}", name=f"v{m}")
                    for m in range(NMC)]
            woT_sb = persist.tile([P, NI, d_model], FP32R, tag="woT")
            wq_sb = persist.tile([P, ND, inner], BF16, tag="wq")
            wk_sb = persist.tile([P, ND, inner], BF16, tag="wk")
            wv_sb = persist.tile([P, ND, inner], BF16, tag="wv")
            ones = persist.tile([P, P], FP32, tag="ones")
            E_sb = persist.tile([P, P], FP32R, tag="E")
            scratch = persist.tile([P, nt], FP32, tag="scratch")
            warm = persist.tile([P, 16], FP32, tag="warm")
            ssb = [persist.tile([P, nt], FP32, tag=f"ssb{p}", name=f"ssb{p}") for p in range(2)]
            rs32 = [persist.tile([P, nt], FP32, tag=f"rs32{p}", name=f"rs32{p}") for p in range(2)]
            rs_p = [persist.tile([P, nt], FP32R, tag=f"rsp{p}", name=f"rsp{p}") for p in range(2)]

            # ---- t=0: exp table preload + const init (overlaps input DMA)
            nc.vector.memset(scratch[:], 0.0)
            nc.scalar.activation(warm[:], scratch[:, 0:16], EXP, scale=1.0)
            nc.vector.memset(ones[:], 1.0)
            nc.vector.tensor_copy(E_sb[:], scratch[:, 0:P])
            nc.vector.tensor_copy(E_sb[0:1, 0:64], ones[0:1, 0:64])
            nc.vector.tensor_copy(E_sb[64:65, 64:128], ones[64:65, 0:64])
            for p2 in range(2):
                nc.vector.memset(ssb[p2][:], 1.0)
                nc.vector.memset(rs32[p2][:], 0.0)
                nc.vector.tensor_copy(rs_p[p2][:], scratch[:])
            for m in range(NMC):
                nc.gpsimd.memset(v_sb[m][:], 1.0)

            # ---- input DMAs in consumption order (pool slots pace them)
            nc.sync.dma_start(wk_sb[:], wkT_r[:, :, :])
            ck = []
            for mt in range(NMT):
                t = ctxp.tile([P, ND, nt], BF16, tag="ck", name=f"ck{mt}")
                ck.append(t)
            nc.sync.dma_start(ck[0][:], ctxT_r[:, :, 0:nt])
            nc.sync.dma_start(wq_sb[:], wqT_r[:, :, :])
            xq = [xqp.tile([P, ND, nt], BF16, tag="xq", name=f"xq{n}") for n in range(NNT)]
            nc.sync.dma_start(xq[0][:], xT_r[:, :, 0:nt])
            nc.sync.dma_start(wv_sb[:], wvT_r[:, :, :])
            for mt in range(1, NMT):
                nc.sync.dma_start(ck[mt][:], ctxT_r[:, :, mt * nt:(mt + 1) * nt])
            nc.sync.dma_start(woT_sb[:], woT_r[:, :, :])
            for n in range(1, NNT):
                nc.sync.dma_start(xq[n][:], xT_r[:, :, n * nt:(n + 1) * nt])

            # ---------------- building blocks ----------------
            def kproj(mt):
                msl = slice(mt * nt, (mt + 1) * nt)
                for i in range(NI):
                    ps = trans.tile([P, nt], FP32, tag="tr")
                    for d in range(ND):
                        nc.tensor.matmul(
                            ps[:], wk_sb[:, d, i * P:(i + 1) * P],
                            ck[mt][:, d, :],
                            start=(d == 0), stop=(d == ND - 1))
                    nc.vector.tensor_copy(kT[i][:, msl], ps[:])

            def qproj(n):
                for i in range(NI):
                    ps = trans.tile([P, nt], FP32, tag="tr")
                    for d in range(ND):
                        nc.tensor.matmul(
                            ps[:], wq_sb[:, d, i * P:(i + 1) * P],
                            xq[n][:, d, :],
                            start=(d == 0), stop=(d == ND - 1))
                    nc.vector.tensor_copy(qT[i][n][:], ps[:])

            def vproj(mt):
                for sl in range(nt // P):
                    m = mt * (nt // P) + sl
                    ps = trans.tile([P, inner], FP32, tag="tr")
                    for d in range(ND):
                        nc.tensor.matmul(
                            ps[:], ck[mt][:, d, sl * P:(sl + 1) * P],
                            wv_sb[:, d, :],
                            start=(d == 0), stop=(d == ND - 1))
                    for h in range(n_heads):
                        nc.vector.tensor_copy(
                            v_sb[m][:, h * 65:h * 65 + 64],
                            ps[:, h * d_head:(h + 1) * d_head])

            def scores_exp(n, hp, mp):
                """Returns (ex0, ex1) bf16 [P, 2*nt] exp-score tiles."""
                nsl = slice(0, nt)
                pq0 = psqp.tile([P, 2 * nt], FP32, tag="psq")
                pq1 = psqp.tile([P, 2 * nt], FP32, tag="psq")
                for s in range(2):
                    m = 2 * mp + s
                    msl = slice(m * P, (m + 1) * P)
                    ssl = slice(s * nt, (s + 1) * nt)
                    nc.tensor.matmul(
                        pq0[:, ssl], kT[hp][0:64, msl], qT[hp][n][0:64, nsl],
                        start=True, stop=True, tile_position=(0, 0))
                    nc.tensor.matmul(
                        pq1[:, ssl], kT[hp][64:128, msl], qT[hp][n][64:128, nsl],
                        start=True, stop=True, tile_position=(64, 0))
                ex0 = expp.tile([P, 2 * nt], BF16, tag="ex")
                ex1 = expp.tile([P, 2 * nt], BF16, tag="ex")
                nc.scalar.activation(ex0[:], pq0[:, :], EXP, scale=scale)
                nc.scalar.activation(ex1[:], pq1[:, :], EXP, scale=scale)
                return ex0, ex1

            def av(hp, mp, U, ex0, ex1):
                h0, h1 = 2 * hp, 2 * hp + 1
                for s in range(2):
                    m = 2 * mp + s
                    ssl = slice(s * nt, (s + 1) * nt)
                    first = (mp == 0 and s == 0)
                    last = (mp == MPAIRS - 1 and s == 1)
                    nc.tensor.matmul(
                        U[0][:, :], v_sb[m][:, h0 * 65:h0 * 65 + 65],
                        ex0[:, ssl], start=first, stop=last)
                    nc.tensor.matmul(
                        U[1][:, :], v_sb[m][:, h1 * 65:h1 * 65 + 65],
                        ex1[:, ssl], start=first, stop=last)

            def norm(n, hp, U, A_sb):
                """A[hp] rows 0:64 = U0[0:64]/S0, rows 64:128 = U1[0:64]/S1."""
                p2 = (2 * n + hp) % 2
                nc.vector.tensor_copy(ssb[p2][0:1, :], U[0][64:65, :])
                nc.vector.tensor_copy(ssb[p2][64:65, :], U[1][64:65, :])
                with nc.allow_low_precision(reason="softmax denom recip"):
                    nc.vector.reciprocal_approx_fast(
                        rs32[p2][0:65, :], ssb[p2][0:65, :])
                nc.vector.tensor_copy(rs_p[p2][0:65, :], rs32[p2][0:65, :])
                bps = trans.tile([P, nt], FP32, tag="tr")
                nc.tensor.matmul(bps[:], E_sb[:], rs_p[p2][:],
                                 start=True, stop=True)
                bsb = normp.tile([P, nt], FP32, tag="bsb")
                nc.vector.tensor_copy(bsb[:], bps[:])
                nc.vector.tensor_mul(A_sb[hp][0:64, :], U[0][0:64, :],
                                     bsb[0:64, :])
                nc.vector.tensor_mul(A_sb[hp][64:128, :], U[1][0:64, :],
                                     bsb[64:128, :])

            def stage4(n):
                A_sb = A_tiles[n]
                for q in range(nt // P):
                    qsl = slice(q * P, (q + 1) * P)
                    for j in range(NJ):
                        jsl = slice(j * nt, (j + 1) * nt)
                        yps = trans.tile([P, nt], FP32, tag="tr")
                        for c in range(NI):
                            nc.tensor.matmul(
                                yps[:], A_sb[c][:, qsl], woT_sb[:, c, jsl],
                                start=(c == 0), stop=(c == NI - 1))
                        ysb = ysbp.tile([P, nt], FP32, tag="ysb")
                        nc.vector.tensor_copy(ysb[:], yps[:])
                        nc.sync.dma_start(
                            y.ap()[n * nt + q * P:n * nt + (q + 1) * P, jsl],
                            ysb[:])

            # ---------------- emission ----------------
            A_tiles = [[apool.tile([P, nt], FP32R, tag="A", name=f"A{n}_{c}")
                        for c in range(NI)] for n in range(NNT)]

            # --- n-tile 0, head-pair 0: interleaved with projections
            kproj(0)
            qproj(0)
            U0 = [upool.tile([65, nt], FP32, tag="U", name=f"U0_0_{u}")
                  for u in range(2)]
            for mt in range(NMT):
                if mt > 0:
                    kproj(mt)
                exs = []
                for mp in (2 * mt, 2 * mt + 1):
                    exs.append(scores_exp(0, 0, mp))
                vproj(mt)
                for k, mp in enumerate((2 * mt, 2 * mt + 1)):
                    av(0, mp, U0, *exs[k])
            norm(0, 0, U0, A_tiles[0])

            def hp_block(n, hp):
                U = [upool.tile([65, nt], FP32, tag="U", name=f"U{n}_{hp}_{u}")
                     for u in range(2)]
                for mp in range(MPAIRS):
                    ex0, ex1 = scores_exp(n, hp, mp)
                    av(hp, mp, U, ex0, ex1)
                return U

            qproj(1)
            U = hp_block(0, 1)
            norm(0, 1, U, A_tiles[0])
            stage4(0)

            for n in range(1, NNT):
                U = hp_block(n, 0)
                norm(n, 0, U, A_tiles[n])
                if n < NNT - 1:
                    qproj(n + 1)
                U = hp_block(n, 1)
                norm(n, 1, U, A_tiles[n])
                stage4(n)

    nc.compile()
    return nc


def shard_inputs(x, context, Wq, Wk, Wv, Wo):
    """Per-core input dicts: core c -> (batch c//4, head-group c%4)."""
    in_maps = []
    for c in range(N_CORES):
        b, g = c // 4, c % 4
        rows = slice(g * 256, (g + 1) * 256)
        bf = ml_dtypes.bfloat16
        in_maps.append({
            "xT": np.ascontiguousarray(x[b].T).astype(bf),
            "ctxT": np.ascontiguousarray(context[b].T).astype(bf),
            "wqT": np.ascontiguousarray(Wq[rows].T).astype(bf),
            "wkT": np.ascontiguousarray(Wk[rows].T).astype(bf),
            "wvT": np.ascontiguousarray(Wv[rows].T).astype(bf),
            "woT": np.ascontiguousarray(Wo[:, rows].T),
        })
    return in_maps


_CACHE = {}


def _get_nc():
    if "nc" not in _CACHE:
        _CACHE["nc"] = build_nc()
    return _CACHE["nc"]


def kernel(x, context, Wq, Wk, Wv, Wo, bo, _trace=False):
    from concourse.bass_utils import run_bass_kernel_spmd

    x = np.asarray(x, dtype=np.float32)
    context = np.asarray(context, dtype=np.float32)
    in_maps = shard_inputs(x, context,
                           np.asarray(Wq, np.float32), np.asarray(Wk, np.float32),
                           np.asarray(Wv, np.float32), np.asarray(Wo, np.float32))
    nc = _get_nc()
    res = run_bass_kernel_spmd(nc, in_maps, core_ids=list(range(N_CORES)),
                               trace=_trace)
    B, N, _ = x.shape
    out = np.zeros((B, N, D_MODEL), dtype=np.float32)
    for c in range(N_CORES):
        out[c // 4] += res.results[c]["y"]
    out += np.asarray(bo, np.float32)[None, None, :]
    if _trace:
        _CACHE["last_results"] = res
    return out


# revision 6
# speedup vs baseline: 1.5739x; 1.0897x over previous
"""CrossAttention Trainium2 kernel (v3 — deferred-AV pipeline).

Full inputs -> shard over 8 cores (batch x head-group) -> Bass kernel ->
host gather (sum head-group partials per batch + bias).

Per-core layout (B=2 batches x 4 head-groups of 4 heads):
  xT    [1024, 2048]  x[b].T
  ctxT  [1024, 2048]  context[b].T
  wqT   [1024, 256]   Wq[rows(g)].T      (rows(g) = g*256 : (g+1)*256)
  wkT   [1024, 256]
  wvT   [1024, 256]
  woT   [256, 1024]   Wo[:, rows(g)].T
  out y [2048, 1024]  partial (sum over g gives batch output; bias on host)

The scalar engine's exp stream (128 ACTIVATE of 128x1024, ~147us) is the
roofline; everything else must hide under it.  Attention runs as 8
sub-blocks k=(n,hp).  Sub-block k's scores+exp are emitted one sub-block
EARLY and its AV matmuls one sub-block LATE (ex tiles stashed in a
32-slot pool), so the exp stream is gated only by the 2-slot scores-PSUM
rotation + cheap score matmuls -- never by the U-accumulator pool or the
softmax-normalization chain.  k=0/1 fuse with the k/v/q projections
(ctx read once for both K and V); a t=0 dummy exp preloads the ACT table
and a t=0 matmul burst pre-warms the PE HAM clock gate.
PSUM: scores 2x[128,1024]f32 (4 banks) + U 2x[65,512] (2) + shared
2-slot transient pool (projections/E/stage4) = 8 banks exactly.
"""
import numpy as np
import ml_dtypes

HEADS = 16
DIM_HEAD = 64
D_MODEL = 1024
N_CORES = 8


def build_nc(n_q=2048, n_kv=2048, d_model=1024, n_heads=4, d_head=64, nt=512):
    """Build the per-core Bass module."""
    import concourse.bass as bass
    import concourse.mybir as mybir
    import concourse.tile as tile
    from concourse import bacc

    FP32 = mybir.dt.float32
    FP32R = mybir.dt.float32r
    BF16 = mybir.dt.bfloat16
    EXP = mybir.ActivationFunctionType.Exp
    P = 128

    inner = n_heads * d_head          # 256
    ND = d_model // P                 # d-chunks (8)
    NI = inner // P                   # i-chunks (2)
    NNT = n_q // nt                   # query tiles (4)
    NMT = n_kv // nt                  # key tiles (4)
    NMC = n_kv // P                   # key chunks (16)
    NJ = d_model // nt                # output col tiles (2)
    NHP = n_heads // 2                # head pairs (2)
    MPAIRS = NMC // 2                 # m-chunk pairs (8)
    NSB = NNT * NHP                   # sub-blocks (8)

    nc = bacc.Bacc(None, target_bir_lowering=False, debug=False)

    xT = nc.dram_tensor("xT", [d_model, n_q], BF16, kind="ExternalInput")
    ctxT = nc.dram_tensor("ctxT", [d_model, n_kv], BF16, kind="ExternalInput")
    wqT = nc.dram_tensor("wqT", [d_model, inner], BF16, kind="ExternalInput")
    wkT = nc.dram_tensor("wkT", [d_model, inner], BF16, kind="ExternalInput")
    wvT = nc.dram_tensor("wvT", [d_model, inner], BF16, kind="ExternalInput")
    woT = nc.dram_tensor("woT", [inner, d_model], FP32R, kind="ExternalInput")
    y = nc.dram_tensor("y", [n_q, d_model], FP32, kind="ExternalOutput")

    xT_r = xT.ap().rearrange("(c p) n -> p c n", p=P)      # [128, ND, n_q]
    ctxT_r = ctxT.ap().rearrange("(c p) m -> p c m", p=P)  # [128, ND, n_kv]
    wqT_r = wqT.ap().rearrange("(c p) i -> p c i", p=P)    # [128, ND, inner]
    wkT_r = wkT.ap().rearrange("(c p) i -> p c i", p=P)
    wvT_r = wvT.ap().rearrange("(c p) i -> p c i", p=P)
    woT_r = woT.ap().rearrange("(c p) j -> p c j", p=P)    # [128, NI, d_model]

    scale = float(d_head) ** -0.5

    with tile.TileContext(nc) as tc:
        with (
            tc.tile_pool(name="persist", bufs=1) as persist,
            tc.tile_pool(name="ctxp", bufs=3) as ctxp,
            tc.tile_pool(name="xqp", bufs=2) as xqp,
            tc.tile_pool(name="expp", bufs=32) as expp,
            tc.tile_pool(name="apool", bufs=2 * NI) as apool,
            tc.tile_pool(name="ysbp", bufs=3) as ysbp,
            tc.tile_pool(name="normp", bufs=2) as normp,
            tc.tile_pool(name="psq", bufs=2, space="PSUM") as psqp,
            tc.tile_pool(name="upool", bufs=2, space="PSUM") as upool,
            tc.tile_pool(name="trans", bufs=2, space="PSUM") as trans,
        ):
            # ---------------- persistent tiles ----------------
            qT = [[persist.tile([P, nt], BF16, tag=f"qT{i}_{n}", name=f"qT{i}_{n}")
                   for n in range(NNT)] for i in range(NI)]
            kT = [persist.tile([P, n_kv], BF16, tag=f"kT{i}", name=f"kT{i}")
                  for i in range(NI)]
            v_sb = [persist.tile([P, n_heads * 65], BF16, tag=f"v{m}", name=f"v{m}")
                    for m in range(NMC)]
            woT_sb = persist.tile([P, NI, d_model], FP32R, tag="woT")
            wq_sb = persist.tile([P, ND, inner], BF16, tag="wq")
            wk_sb = persist.tile([P, ND, inner], BF16, tag="wk")
            wv_sb = persist.tile([P, ND, inner], BF16, tag="wv")
            ones = persist.tile([P, P], FP32, tag="ones")
            E_sb = persist.tile([P, P], FP32R, tag="E")
            scratch = persist.tile([P, nt], FP32, tag="scratch")
            warm = persist.tile([P, 16], FP32, tag="warm")
            wsrc = persist.tile([P, P], BF16, tag="wsrc")
            ssb = [persist.tile([P, nt], FP32, tag=f"ssb{p}", name=f"ssb{p}")
                   for p in range(2)]
            rs32 = [persist.tile([P, nt], FP32, tag=f"rs32{p}", name=f"rs32{p}")
                    for p in range(2)]
            rs_p = [persist.tile([P, nt], FP32R, tag=f"rsp{p}", name=f"rsp{p}")
                    for p in range(2)]

            # ---- t=0: exp table preload + const init (overlaps input DMA)
            nc.vector.memset(scratch[:], 0.0)
            nc.scalar.activation(warm[:], scratch[:, 0:16], EXP, scale=1.0)
            nc.vector.memset(ones[:], 1.0)
            nc.vector.tensor_copy(E_sb[:], scratch[:, 0:P])
            nc.vector.tensor_copy(E_sb[0:1, 0:64], ones[0:1, 0:64])
            nc.vector.tensor_copy(E_sb[64:65, 64:128], ones[64:65, 0:64])
            for p2 in range(2):
                nc.vector.memset(ssb[p2][:], 1.0)
                nc.vector.memset(rs32[p2][:], 0.0)
                nc.vector.tensor_copy(rs_p[p2][:], scratch[:])
            for m in range(NMC):
                nc.gpsimd.memset(v_sb[m][:], 1.0)
            # PE warm-up: ~5us of dummy matmuls trips the HAM clock gate to
            # 8/8 before the first projection.
            nc.vector.tensor_copy(wsrc[:], scratch[:, 0:P])
            wps = trans.tile([P, nt], FP32, tag="tr", name="warmps")
            for w in range(48):
                nc.tensor.matmul(wps[:, 0:P], wsrc[:], wsrc[:],
                                 start=True, stop=True)

            # ---- input DMAs in consumption order (pool slots pace them)
            nc.sync.dma_start(wk_sb[:], wkT_r[:, :, :])
            ck = [ctxp.tile([P, ND, nt], BF16, tag="ck", name=f"ck{mt}")
                  for mt in range(NMT)]
            nc.sync.dma_start(ck[0][:], ctxT_r[:, :, 0:nt])
            nc.sync.dma_start(wq_sb[:], wqT_r[:, :, :])
            xq = [xqp.tile([P, ND, nt], BF16, tag="xq", name=f"xq{n}")
                  for n in range(NNT)]
            nc.sync.dma_start(xq[0][:], xT_r[:, :, 0:nt])
            nc.sync.dma_start(wv_sb[:], wvT_r[:, :, :])
            for mt in range(1, NMT):
                nc.sync.dma_start(ck[mt][:], ctxT_r[:, :, mt * nt:(mt + 1) * nt])
            nc.sync.dma_start(woT_sb[:], woT_r[:, :, :])
            for n in range(1, NNT):
                nc.sync.dma_start(xq[n][:], xT_r[:, :, n * nt:(n + 1) * nt])

            # ---------------- building blocks ----------------
            def kproj(mt):
                msl = slice(mt * nt, (mt + 1) * nt)
                for i in range(NI):
                    ps = trans.tile([P, nt], FP32, tag="tr")
                    for d in range(ND):
                        nc.tensor.matmul(
                            ps[:], wk_sb[:, d, i * P:(i + 1) * P],
                            ck[mt][:, d, :],
                            start=(d == 0), stop=(d == ND - 1))
                    nc.vector.tensor_copy(kT[i][:, msl], ps[:])

            def qproj(n, i):
                ps = trans.tile([P, nt], FP32, tag="tr")
                for d in range(ND):
                    nc.tensor.matmul(
                        ps[:], wq_sb[:, d, i * P:(i + 1) * P],
                        xq[n][:, d, :],
                        start=(d == 0), stop=(d == ND - 1))
                nc.vector.tensor_copy(qT[i][n][:], ps[:])

            def vproj(mt):
                for sl in range(nt // P):
                    m = mt * (nt // P) + sl
                    ps = trans.tile([P, inner], FP32, tag="tr")
                    for d in range(ND):
                        nc.tensor.matmul(
                            ps[:], ck[mt][:, d, sl * P:(sl + 1) * P],
                            wv_sb[:, d, :],
                            start=(d == 0), stop=(d == ND - 1))
                    for h in range(n_heads):
                        nc.vector.tensor_copy(
                            v_sb[m][:, h * 65:h * 65 + 64],
                            ps[:, h * d_head:(h + 1) * d_head])

            def scores_exp(n, hp, mp):
                """Returns (ex0, ex1): bf16 [P, 2*nt] exp-score tiles."""
                pq0 = psqp.tile([P, 2 * nt], FP32, tag="psq")
                pq1 = psqp.tile([P, 2 * nt], FP32, tag="psq")
                for s in range(2):
                    m = 2 * mp + s
                    msl = slice(m * P, (m + 1) * P)
                    ssl = slice(s * nt, (s + 1) * nt)
                    nc.tensor.matmul(
                        pq0[:, ssl], kT[hp][0:64, msl], qT[hp][n][0:64, :],
                        start=True, stop=True, tile_position=(0, 0))
                    nc.tensor.matmul(
                        pq1[:, ssl], kT[hp][64:128, msl], qT[hp][n][64:128, :],
                        start=True, stop=True, tile_position=(64, 0))
                ex0 = expp.tile([P, 2 * nt], BF16, tag="ex")
                ex1 = expp.tile([P, 2 * nt], BF16, tag="ex")
                nc.scalar.activation(ex0[:], pq0[:, :], EXP, scale=scale)
                nc.scalar.activation(ex1[:], pq1[:, :], EXP, scale=scale)
                return ex0, ex1

            def av(hp, mp, U, ex0, ex1):
                h0, h1 = 2 * hp, 2 * hp + 1
                for s in range(2):
                    m = 2 * mp + s
                    ssl = slice(s * nt, (s + 1) * nt)
                    first = (mp == 0 and s == 0)
                    last = (mp == MPAIRS - 1 and s == 1)
                    nc.tensor.matmul(
                        U[0][:, :], v_sb[m][:, h0 * 65:h0 * 65 + 65],
                        ex0[:, ssl], start=first, stop=last)
                    nc.tensor.matmul(
                        U[1][:, :], v_sb[m][:, h1 * 65:h1 * 65 + 65],
                        ex1[:, ssl], start=first, stop=last)

            # ---------------- emission: deferred-AV pipeline ----------------
            A_tiles = [[apool.tile([P, nt], FP32R, tag="A", name=f"A{n}_{c}")
                        for c in range(NI)] for n in range(NNT)]

            def norm(k, U):
                """A[hp] rows 0:64 = U0[0:64]/S0, rows 64:128 = U1[0:64]/S1.

                U is staged out to SBUF first so its PSUM slots free early.
                """
                n, hp = k // NHP, k % NHP
                A_sb = A_tiles[n]
                p2 = k % 2
                usb = normp.tile([P, nt], FP32, tag="usb")
                nc.vector.tensor_copy(ssb[p2][0:1, :], U[0][64:65, :])
                nc.vector.tensor_copy(usb[0:64, :], U[0][0:64, :])
                nc.vector.tensor_copy(ssb[p2][64:65, :], U[1][64:65, :])
                nc.vector.tensor_copy(usb[64:128, :], U[1][0:64, :])
                with nc.allow_low_precision(reason="softmax denom recip"):
                    nc.vector.reciprocal_approx_fast(
                        rs32[p2][0:65, :], ssb[p2][0:65, :])
                nc.vector.tensor_copy(rs_p[p2][0:65, :], rs32[p2][0:65, :])
                bps = trans.tile([P, nt], FP32, tag="tr")
                nc.tensor.matmul(bps[:], E_sb[:], rs_p[p2][:],
                                 start=True, stop=True)
                bsb = normp.tile([P, nt], FP32, tag="bsb")
                nc.vector.tensor_copy(bsb[:], bps[:])
                nc.vector.tensor_mul(A_sb[hp][0:64, :], usb[0:64, :],
                                     bsb[0:64, :])
                nc.vector.tensor_mul(A_sb[hp][64:128, :], usb[64:128, :],
                                     bsb[64:128, :])

            def stage4(n):
                A_sb = A_tiles[n]
                for q in range(nt // P):
                    qsl = slice(q * P, (q + 1) * P)
                    for j in range(NJ):
                        jsl = slice(j * nt, (j + 1) * nt)
                        yps = trans.tile([P, nt], FP32, tag="tr")
                        for c in range(NI):
                            nc.tensor.matmul(
                                yps[:], A_sb[c][:, qsl], woT_sb[:, c, jsl],
                                start=(c == 0), stop=(c == NI - 1))
                        ysb = ysbp.tile([P, nt], FP32, tag="ysb")
                        nc.vector.tensor_copy(ysb[:], yps[:])
                        nc.sync.dma_start(
                            y.ap()[n * nt + q * P:n * nt + (q + 1) * P, jsl],
                            ysb[:])

            # Sub-block k = (n, hp) = (k//2, k%2).  exs[k][mp] stashes the
            # exp tiles of sub-block k until its AVs run one sub-block later.
            exs = [[None] * MPAIRS for _ in range(NSB)]
            Us = [None] * NSB

            def new_U(k):
                Us[k] = [upool.tile([65, nt], FP32, tag="U", name=f"U{k}_{u}")
                         for u in range(2)]

            # --- head: projections + sub-block 0 (scores+exp+AV inline)
            #     + sub-block 1 (scores+exp stashed), per ctx m-tile.
            kproj(0)
            qproj(0, 0)
            qproj(0, 1)
            new_U(0)
            for mt in range(NMT):
                if mt > 0:
                    kproj(mt)
                mps = (2 * mt, 2 * mt + 1)
                for mp in mps:
                    exs[0][mp] = scores_exp(0, 0, mp)
                vproj(mt)
                for mp in mps:
                    av(0, mp, Us[0], *exs[0][mp])
                for mp in mps:
                    exs[1][mp] = scores_exp(0, 1, mp)
                if mt == 2:
                    qproj(1, 0)
                if mt == 3:
                    qproj(1, 1)
            norm(0, Us[0])

            # --- steady pipeline: unit j emits sub-block j+1's scores/exp
            #     interleaved with sub-block j's deferred AVs.
            for j in range(1, NSB):
                n_next, hp_next = (j + 1) // NHP, (j + 1) % NHP
                new_U(j)
                for mp in range(MPAIRS):
                    if j + 1 < NSB:
                        exs[j + 1][mp] = scores_exp(n_next, hp_next, mp)
                    av(j % NHP, mp, Us[j], *exs[j][mp])
                    exs[j][mp] = None
                norm(j, Us[j])
                if j == 2:
                    qproj(2, 0)
                    qproj(2, 1)
                if j == 4:
                    qproj(3, 0)
                    qproj(3, 1)
                if j % NHP == 1:
                    stage4(j // NHP)

    nc.compile()
    return nc


def shard_inputs(x, context, Wq, Wk, Wv, Wo):
    """Per-core input dicts: core c -> (batch c//4, head-group c%4)."""
    in_maps = []
    for c in range(N_CORES):
        b, g = c // 4, c % 4
        rows = slice(g * 256, (g + 1) * 256)
        bf = ml_dtypes.bfloat16
        in_maps.append({
            "xT": np.ascontiguousarray(x[b].T).astype(bf),
            "ctxT": np.ascontiguousarray(context[b].T).astype(bf),
            "wqT": np.ascontiguousarray(Wq[rows].T).astype(bf),
            "wkT": np.ascontiguousarray(Wk[rows].T).astype(bf),
            "wvT": np.ascontiguousarray(Wv[rows].T).astype(bf),
            "woT": np.ascontiguousarray(Wo[:, rows].T),
        })
    return in_maps


_CACHE = {}


def _get_nc():
    if "nc" not in _CACHE:
        _CACHE["nc"] = build_nc()
    return _CACHE["nc"]


def kernel(x, context, Wq, Wk, Wv, Wo, bo, _trace=False):
    from concourse.bass_utils import run_bass_kernel_spmd

    x = np.asarray(x, dtype=np.float32)
    context = np.asarray(context, dtype=np.float32)
    in_maps = shard_inputs(x, context,
                           np.asarray(Wq, np.float32), np.asarray(Wk, np.float32),
                           np.asarray(Wv, np.float32), np.asarray(Wo, np.float32))
    nc = _get_nc()
    res = run_bass_kernel_spmd(nc, in_maps, core_ids=list(range(N_CORES)),
                               trace=_trace)
    B, N, _ = x.shape
    out = np.zeros((B, N, D_MODEL), dtype=np.float32)
    for c in range(N_CORES):
        out[c // 4] += res.results[c]["y"]
    out += np.asarray(bo, np.float32)[None, None, :]
    if _trace:
        _CACHE["last_results"] = res
    return out
